# revision 1
# baseline (speedup 1.0000x reference)
"""CRF loss (forward-algorithm log-partition minus gold path score, batch mean)
on 8 Trainium2 NeuronCores.

Strategy (data-parallel over batch, 64 rows/core, identical SPMD program):
  Denominator: linear-space forward recursion alpha_{t+1} = exp(emit-c) (*) (E @ alpha_t)
    in [T=48 partitions, batch free] layout; one PE matmul + one DVE tensor_tensor
    per step, two 32-row interleaved chains; periodic per-row rescaling; per-step
    z_t = endexp^T alpha_t extraction into a [128,512] z-buffer; the row's logZ is
    selected at t = len-1 with a last-step mask dot (data independent).
  Numerator: one-hot match masks (iota is_equal, sentinel tags for masking) feed
    PSUM-accumulated matmuls: trace(match^T @ emis), <bigram-hist, transitions>,
    start/end histograms. Only the batch mean is needed, so gathers collapse
    into histograms.
Host only shards/relayouts inputs and sums the 8 per-core partial scalars.
"""

import numpy as np
from contextlib import ExitStack

import concourse.bacc as bacc
import concourse.tile as tile
from concourse import mybir

B, S, T = 512, 1024, 48
NCORES = 8
BC = B // NCORES          # rows per core = 64
W = 32                    # rows per chain (2 chains)
ST = 64                   # time steps per tile
NBLK = S // ST
RESC = 128                # rescale cadence
C_SHIFT = float(np.float32(np.log(T) + 0.5))

f32 = mybir.dt.float32
bf16 = mybir.dt.bfloat16
i32 = mybir.dt.int32
u8 = mybir.dt.uint8
OP = mybir.AluOpType
AF = mybir.ActivationFunctionType


def _build(repeat=1, no_num=False, no_z=False, no_resc=False, no_build=False, no_finals=False, fu=99, swap_tt=False, gp_match=False, nch=2, em32=False, lazy=True):
    """v2: alpha mults write into a wide [T, 8*BC] group buffer so z_t extraction
    is ONE endexp matmul per 8 steps (was 2/step); chain 1's elementwise mult
    runs on Pool (gpsimd) instead of DVE; match masks build in chunks of 8
    alternating DVE/Pool; z rows assemble in a free-dim staging row, one
    SBUF->SBUF DMA per 64-step block."""
    nc = bacc.Bacc(target_bir_lowering=False, debug=False)
    emT_d = nc.dram_tensor("emT32" if em32 else "emT", [T, S * BC], f32 if em32 else bf16, kind="ExternalInput")
    # paired layouts: partition p*64+b <-> (batch b, step 2k+p); contraction
    # dim 128 so accEE/accCO need one matmul per TWO steps
    # numrhs = [emb(50) | matchS(48)] fused -> ONE accumulation matmul per
    # step-pair computes both the emission/start/end histograms and bigrams
    match_d = nc.dram_tensor("matchh", [2 * BC, (S // 2) * T], bf16, kind="ExternalInput")
    numrhs_d = nc.dram_tensor("numrhsh", [2 * BC, (S // 2) * 98], bf16, kind="ExternalInput")
    len_d = nc.dram_tensor("lens", [BC, 1], f32, kind="ExternalInput")
    lastw_d = nc.dram_tensor("lastwh", [128, 512], bf16, kind="ExternalInput")
    transT_d = nc.dram_tensor("transT", [T, T], f32, kind="ExternalInput")
    trans_d = nc.dram_tensor("trans", [T, T], f32, kind="ExternalInput")
    start_d = nc.dram_tensor("start", [T, 1], f32, kind="ExternalInput")
    end_d = nc.dram_tensor("end", [T, 1], f32, kind="ExternalInput")
    out_d = nc.dram_tensor("out", [1, 8], f32, kind="ExternalOutput")

    with tile.TileContext(nc) as tc, ExitStack() as ctx:
        consts = ctx.enter_context(tc.tile_pool(name="consts", bufs=1))
        rawp = ctx.enter_context(tc.tile_pool(name="rawp", bufs=2))
        dp = ctx.enter_context(tc.tile_pool(name="dp", bufs=2))
        mp = ctx.enter_context(tc.tile_pool(name="mp", bufs=2))
        nrp = ctx.enter_context(tc.tile_pool(name="nrp", bufs=2))
        ap = ctx.enter_context(tc.tile_pool(name="ap", bufs=3))
        sm = ctx.enter_context(tc.tile_pool(name="sm", bufs=2))
        awp = ctx.enter_context(tc.tile_pool(name="awp", bufs=3))
        pers = ctx.enter_context(tc.tile_pool(name="pers", bufs=1))
        cps = ctx.enter_context(tc.tile_pool(name="cps", bufs=1, space="PSUM"))
        zps = ctx.enter_context(tc.tile_pool(name="zps", bufs=(1 if nch > 2 else 2), space="PSUM"))
        acps = ctx.enter_context(tc.tile_pool(name="acps", bufs=1, space="PSUM"))
        tps = ctx.enter_context(tc.tile_pool(name="tps", bufs=1, space="PSUM"))

        # ---- constants ----
        transT = consts.tile([T, T], f32)
        nc.sync.dma_start(out=transT, in_=transT_d[:, :])
        trans_sb = consts.tile([T, T], f32)
        nc.sync.dma_start(out=trans_sb, in_=trans_d[:, :])
        start_sb = consts.tile([T, 1], f32)
        nc.sync.dma_start(out=start_sb, in_=start_d[:, :])
        end_sb = consts.tile([T, 1], f32)
        nc.sync.dma_start(out=end_sb, in_=end_d[:, :])

        b0 = consts.tile([T, 1], f32)
        nc.vector.memset(b0, 0.0)
        b0_64 = consts.tile([BC, 1], f32)
        nc.vector.memset(b0_64, 0.0)
        biasmc = consts.tile([T, 1], f32)
        nc.vector.memset(biasmc, -C_SHIFT)
        startmc = consts.tile([T, 1], f32)
        nc.vector.tensor_scalar_add(startmc, start_sb, -C_SHIFT)

        ET = consts.tile([T, T], bf16)
        nc.scalar.activation(ET, transT, AF.Exp, bias=b0[:, :])
        endexp = consts.tile([T, 1], bf16)
        nc.scalar.activation(endexp, end_sb, AF.Exp, bias=b0[:, :])

        iota48f = consts.tile([T, T], f32)
        nc.gpsimd.iota(iota48f, pattern=[[1, T]], base=0, channel_multiplier=0,
                       allow_small_or_imprecise_dtypes=True)
        iotacolf = consts.tile([T, 1], f32)
        nc.gpsimd.iota(iotacolf, pattern=[[0, 1]], base=0, channel_multiplier=1,
                       allow_small_or_imprecise_dtypes=True)
        ident48 = consts.tile([T, T], f32)
        nc.vector.tensor_scalar(ident48, iota48f, iotacolf[:, :], None, op0=OP.is_equal)

        ones1 = consts.tile([1, 1], f32)
        nc.vector.memset(ones1, 1.0)
        onesProw = consts.tile([1, T], bf16)
        nc.vector.memset(onesProw, 1.0)
        onescol48b = consts.tile([T, 1], bf16)
        nc.vector.memset(onescol48b, 1.0)
        onesP = consts.tile([BC, 1], f32)
        nc.vector.memset(onesP, 1.0)
        ones128 = consts.tile([128, 1], f32)
        nc.vector.memset(ones128, 1.0)

        # ---- per-batch-row static prep (match/emb/lastw precomputed on host) ----
        lencol = consts.tile([BC, 1], f32)
        nc.sync.dma_start(out=lencol, in_=len_d[:, :])

        lastw = consts.tile([128, 512], bf16)
        nc.sync.dma_start(out=lastw, in_=lastw_d[:, :])

        def body(_iv):
            zbuf = pers.tile([128, 512], f32, tag="zbuf")
            if no_z:
                nc.vector.memset(zbuf, 1.0)
            capC = sm.tile([BC, 1], f32, tag="capC")
            nc.vector.memset(capC, 0.0)

            alphas = [None, None]
            acc = acps.tile([T, 98], f32, tag="acc")
            accEE = acc[:, 0:50]
            accCO = acc[:, 50:98]
            if no_num:
                nc.vector.memset(acc, 1.0)
            aw = None
            resc_pending = None

            # deferred PE work queue: bookkeeping matmuls drain behind the
            # alpha matmuls at a bounded rate per step so no single step's PE
            # load exceeds the DVE window
            from collections import deque
            peq = deque()  # (est_cost_ns, thunk)
            NPAIR = S // 2

            def _num_thunk(mt, nrt, kl, kg):
                def th():
                    nc.tensor.matmul(acc, lhsT=mt[:, kl, :], rhs=nrt[:, kl, :],
                                     start=(kg == 0), stop=(kg == NPAIR - 1), skip_group_check=True)
                return th

            def _z_thunks(awt, g):
                cell = {}

                def th1():
                    zl = zps.tile([1, 512], f32, tag="zline", name="zline")
                    cell["zl"] = zl
                    nc.tensor.matmul(zl[0:1, 0:256], lhsT=endexp, rhs=awt[:, 0:256],
                                     start=True, stop=True, skip_group_check=True)

                def th2():
                    zl = cell["zl"]
                    nc.tensor.matmul(zl[0:1, 256:512], lhsT=endexp, rhs=awt[:, 256:512],
                                     start=True, stop=True, skip_group_check=True)

                def th3():
                    zl = cell["zl"]
                    zrow_sb = sm.tile([1, 512], f32, tag="zrow", bufs=3, name="zrow_sb")
                    nc.scalar.activation(zrow_sb, zl, AF.Copy)
                    nc.sync.dma_start(out=zbuf[g:g + 1, :], in_=zrow_sb)
                return th1, th2, th3

            def flush_pend(budget=150.0):
                spent = 0.0
                while peq and spent < budget:
                    cost, th = peq.popleft()
                    th()
                    spent += cost

            for blk in range(NBLK):
                raw = rawp.tile([T, ST, BC], f32 if em32 else bf16, tag="raw")
                nc.sync.dma_start(out=raw, in_=emT_d[:, blk * ST * BC:(blk + 1) * ST * BC].rearrange("t (s b) -> t s b", b=BC))
                d = dp.tile([T, ST, BC], bf16, tag="d")
                nc.scalar.activation(d, raw, AF.Exp, bias=biasmc[:, :])

                NP = ST // 2  # pairs per block
                if no_build:
                    match = None
                else:
                 match = mp.tile([2 * BC, NP, T], bf16, tag="match")
                 nc.sync.dma_start(out=match, in_=match_d[:, blk * NP * T:(blk + 1) * NP * T].rearrange("b (k t) -> b k t", t=T))
                 numr = nrp.tile([2 * BC, NP, 98], bf16, tag="numr")
                 nc.sync.dma_start(out=numr, in_=numrhs_d[:, blk * NP * 98:(blk + 1) * NP * 98].rearrange("b (k e) -> b k e", e=98))

                for st in range(ST):
                    t = blk * ST + st
                    slot = t & 7
                    if slot == 0:
                        aw = awp.tile([T, 8 * BC], bf16, tag="awide")

                    ws = [BC // nch + (1 if c < BC % nch else 0) for c in range(nch)]
                    off = [sum(ws[:c]) for c in range(nch + 1)]
                    if t == 0:
                        nc.scalar.activation(aw[:, 0:BC], raw[:, 0, :], AF.Exp, bias=startmc[:, :])
                        alphas = [aw[:, off[c]:off[c + 1]] for c in range(nch)]
                    else:
                        for c in range(nch):
                            ps = cps.tile([T, ws[c]], f32, tag=f"cps{c}")
                            nc.tensor.matmul(ps, lhsT=ET, rhs=alphas[c], start=True, stop=True)
                            sl = aw[:, slot * BC + off[c]: slot * BC + off[c + 1]]
                            nc.vector.tensor_tensor(out=sl, in0=ps, in1=d[:, st, off[c]:off[c + 1]], op=OP.mult)
                            alphas[c] = sl

                    # deferred work from the previous step lands behind this
                    # step's alpha matmuls in PE program order
                    flush_pend()

                    if (not no_z) and slot == 7:
                        th1, th2, th3 = _z_thunks(aw, t >> 3)
                        peq.append((215.0, th1))
                        peq.append((215.0, th2))
                        peq.append((0.0, th3))
                    if (not no_num) and (t & 1) == 1:
                        peq.append((200.0, _num_thunk(match, numr, st >> 1, t >> 1)))

                    # lazy-rescale application, two steps after the factor was
                    # launched: alphas scaled via side tiles (aw slots keep the
                    # old scale so z stays consistent with the lenmask cut)
                    if resc_pending is not None and t == resc_pending[2] + 2:
                        rb, recipb, tr = resc_pending
                        resc_pending = None
                        for c in range(nch):
                            a_new = ap.tile([T, ws[c]], bf16, tag=f"alpha{c}")
                            nc.vector.tensor_tensor(out=a_new, in0=alphas[c], in1=rb[:, off[c]:off[c + 1]], op=OP.mult)
                            alphas[c] = a_new
                        # lnrow = ln(1/s) = -ln(s); capC accumulates negated,
                        # finals subtract it
                        lnrow = sm.tile([1, BC], f32, tag="lnrow")
                        nc.scalar.activation(lnrow, recipb, AF.Ln, bias=b0_64[0:1, :])
                        lncol = tps.tile([BC, 1], f32, tag="trow")
                        nc.tensor.matmul(lncol, lhsT=lnrow, rhs=ones1, start=True, stop=True)
                        lenmask = sm.tile([BC, 1], f32, tag="lenmask")
                        nc.vector.tensor_scalar(lenmask, lencol, float(tr + 3), None, op0=OP.is_gt)
                        capC_new = sm.tile([BC, 1], f32, tag="capC")
                        nc.vector.scalar_tensor_tensor(out=capC_new, in0=lncol, scalar=lenmask[:, :], in1=capC, op0=OP.mult, op1=OP.add)
                        capC = capC_new

                    # periodic rescale: launch the factor computation; the
                    # chain keeps running and the scale lands at t+2
                    if t % RESC == 0 and t > 0 and not no_resc:
                        sps = cps.tile([1, BC], f32, tag=("spsx" if lazy else f"cps{min(1, nch - 1)}"))
                        for c in range(nch):
                            nc.tensor.matmul(sps[0:1, off[c]:off[c + 1]], lhsT=onescol48b,
                                             rhs=alphas[c], start=True, stop=True, skip_group_check=True)
                        recipb = sm.tile([1, BC], bf16, tag="recipb")
                        with nc.allow_low_precision(reason="rescale factor; logZ tolerance is ample"):
                            nc.vector.reciprocal(recipb, sps)
                        if lazy:
                            rb = tps.tile([T, BC], f32, tag="trow", name="rb")
                        else:
                            rb = cps.tile([T, BC], f32, tag="cps0", name="rb")
                        nc.tensor.matmul(rb, lhsT=onesProw, rhs=recipb, start=True, stop=True)
                        if lazy:
                            resc_pending = (rb, recipb, t)
                        else:
                            for c in range(nch):
                                a_new = ap.tile([T, ws[c]], bf16, tag=f"alpha{c}")
                                nc.vector.tensor_tensor(out=a_new, in0=alphas[c], in1=rb[:, off[c]:off[c + 1]], op=OP.mult)
                                alphas[c] = a_new
                            lnrow = sm.tile([1, BC], f32, tag="lnrow")
                            nc.scalar.activation(lnrow, recipb, AF.Ln, bias=b0_64[0:1, :])
                            lncol = tps.tile([BC, 1], f32, tag="trow")
                            nc.tensor.matmul(lncol, lhsT=lnrow, rhs=ones1, start=True, stop=True)
                            lenmask = sm.tile([BC, 1], f32, tag="lenmask")
                            nc.vector.tensor_scalar(lenmask, lencol, float(t + 1), None, op0=OP.is_gt)
                            capC_new = sm.tile([BC, 1], f32, tag="capC")
                            nc.vector.scalar_tensor_tensor(out=capC_new, in0=lncol, scalar=lenmask[:, :], in1=capC, op0=OP.mult, op1=OP.add)
                            capC = capC_new

            flush_pend(budget=float("inf"))

            # ---- finals ----
            if no_finals:
                outrow = sm.tile([1, 8], f32, tag="outrow")
                nc.vector.memset(outrow, 0.0)
                nc.vector.tensor_copy(outrow[0:1, 0:1], accEE[0:1, 0:1])
                nc.vector.tensor_copy(outrow[0:1, 1:2], zbuf[0:1, 0:1])
                nc.vector.tensor_copy(outrow[0:1, 2:3], capC[0:1, 0:1])
                nc.sync.dma_start(out=out_d[:, :], in_=outrow)
                return
            outrow = sm.tile([1, 8], f32, tag="outrow")
            nc.vector.memset(outrow, 0.0)
            def _dump():
                nc.sync.dma_start(out=out_d[:, :], in_=outrow)
            prod = sm.tile([128, 512], f32, tag="prod")
            nc.vector.tensor_tensor(out=prod, in0=zbuf, in1=lastw, op=OP.mult)
            colsum = zps.tile([1, 512], f32, tag="zline")
            nc.tensor.matmul(colsum, lhsT=ones128, rhs=prod, start=True, stop=True)
            if fu <= 1:
                nc.vector.tensor_copy(outrow[0:1, 0:1], colsum[0:1, 0:1]); _dump(); return
            zcap = sm.tile([1, BC], f32, tag="zcap")
            nc.vector.tensor_reduce(out=zcap, in_=colsum[0:1, :].rearrange("o (s b) -> o b s", s=8),
                                    op=OP.add, axis=mybir.AxisListType.X)
            zmin = sm.tile([1, 1], f32, tag="zmin")
            nc.vector.tensor_reduce(out=zmin, in_=zcap, op=OP.min, axis=mybir.AxisListType.X)
            nc.vector.tensor_copy(outrow[0:1, 5:6], zmin)
            zmax = sm.tile([1, 1], f32, tag="zmax")
            nc.vector.tensor_reduce(out=zmax, in_=zcap, op=OP.max, axis=mybir.AxisListType.X)
            nc.vector.tensor_copy(outrow[0:1, 6:7], zmax)
            if fu <= 2:
                nc.vector.tensor_copy(outrow[0:1, 0:1], zcap[0:1, 0:1]); _dump(); return
            zcol = tps.tile([BC, 1], f32, tag="trow")
            nc.tensor.matmul(zcol, lhsT=zcap, rhs=ones1, start=True, stop=True)
            lnz = sm.tile([BC, 1], f32, tag="lnz")
            nc.scalar.activation(lnz, zcol, AF.Ln, bias=b0_64[:, :])
            t2 = sm.tile([BC, 1], f32, tag="t2")
            nc.vector.tensor_tensor(out=t2, in0=lnz, in1=capC, op=OP.subtract)
            logZ = sm.tile([BC, 1], f32, tag="logZ")
            nc.vector.scalar_tensor_tensor(out=logZ, in0=lencol, scalar=C_SHIFT, in1=t2, op0=OP.mult, op1=OP.add)
            if fu <= 3:
                nc.vector.tensor_copy(outrow[0:1, 0:1], logZ[0:1, 0:1]); _dump(); return
            sumZ = tps.tile([1, 1], f32, tag="trow")
            nc.tensor.matmul(sumZ, lhsT=logZ, rhs=onesP, start=True, stop=True)
            nc.vector.tensor_copy(outrow[0:1, 0:1], sumZ)
            if fu <= 4:
                _dump(); return

            numcat = sm.tile([T, 4], f32, tag="numcat")
            nc.vector.memset(numcat, 0.0)
            trash1 = sm.tile([T, T], f32, tag="trash1")
            nc.vector.tensor_tensor(out=trash1, in0=accEE[:, 0:T], in1=ident48, op=OP.mult)
            trashb1 = sm.tile([T, T], bf16, tag="trashb1")
            nc.scalar.activation(trashb1, trash1, AF.Copy, accum_out=numcat[:, 0:1])
            trash2 = sm.tile([T, T], f32, tag="trash2")
            nc.vector.tensor_tensor(out=trash2, in0=accCO, in1=trans_sb, op=OP.mult)
            trashb2 = sm.tile([T, T], bf16, tag="trashb2")
            nc.scalar.activation(trashb2, trash2, AF.Copy, accum_out=numcat[:, 1:2])
            nc.vector.tensor_tensor(out=numcat[:, 2:3], in0=accEE[:, T:T + 1], in1=end_sb, op=OP.mult)
            nc.vector.tensor_tensor(out=numcat[:, 3:4], in0=accEE[:, T + 1:T + 2], in1=start_sb, op=OP.mult)
            ones48f = sm.tile([T, 1], f32, tag="ones48f")
            nc.vector.memset(ones48f, 1.0)
            nsum = tps.tile([1, 4], f32, tag="trow")
            nc.tensor.matmul(nsum, lhsT=ones48f, rhs=numcat, start=True, stop=True)

            nc.vector.tensor_copy(outrow[0:1, 1:5], nsum)
            nc.sync.dma_start(out=out_d[:, :], in_=outrow)

        if repeat == 1:
            body(0)
        else:
            with tc.For_i(0, repeat, 1) as iv:
                body(iv)
    nc.compile()
    return nc


class _SpmdRunner:
    def __init__(self, nc, n_cores=NCORES):
        import jax
        from jax.sharding import Mesh, PartitionSpec, NamedSharding
        from jax.experimental.shard_map import shard_map
        from concourse.bass2jax import _bass_exec_p, install_neuronx_cc_hook, partition_id_tensor
        self.jax = jax
        install_neuronx_cc_hook()
        self.nc = nc
        self.n_cores = n_cores
        partition_name = nc.partition_id_tensor.name if nc.partition_id_tensor else None
        in_names, out_names, out_avals, zero_outs = [], [], [], []
        for alloc in nc.m.functions[0].allocations:
            if not isinstance(alloc, mybir.MemoryLocationSet):
                continue
            name = alloc.memorylocations[0].name
            if alloc.kind == "ExternalInput":
                if name != partition_name:
                    in_names.append(name)
            elif alloc.kind == "ExternalOutput":
                shape = tuple(alloc.tensor_shape)
                dtype = mybir.dt.np(alloc.dtype)
                out_names.append(name)
                out_avals.append(jax.core.ShapedArray(shape, dtype))
                zero_outs.append(np.zeros(shape, dtype))
        self.in_names, self.out_names, self.zero_outs = in_names, out_names, zero_outs
        n_params, n_outs = len(in_names), len(out_avals)
        all_in = list(in_names) + list(out_names)
        if partition_name is not None:
            all_in.append(partition_name)

        def _body(*args):
            operands = list(args)
            if partition_name is not None:
                operands.append(partition_id_tensor())
            return tuple(_bass_exec_p.bind(
                *operands, out_avals=tuple(out_avals), in_names=tuple(all_in),
                out_names=tuple(out_names), lowering_input_output_aliases=(),
                sim_require_finite=True, sim_require_nnan=True, nc=nc))

        devices = jax.devices()[:n_cores]
        self.mesh = Mesh(np.asarray(devices), ("core",))
        self.fn = jax.jit(
            shard_map(_body, mesh=self.mesh,
                      in_specs=(PartitionSpec("core"),) * (n_params + n_outs),
                      out_specs=(PartitionSpec("core"),) * n_outs, check_rep=False),
            donate_argnums=tuple(range(n_params, n_params + n_outs)), keep_unused=True)
        self.sharding = NamedSharding(self.mesh, PartitionSpec("core"))

    def put_inputs(self, in_maps):
        concat = [np.concatenate([np.asarray(in_maps[c][n]) for c in range(self.n_cores)], axis=0)
                  for n in self.in_names]
        return [self.jax.device_put(a, self.sharding) for a in concat]

    def __call__(self, dev_inputs):
        zouts = [self.jax.device_put(np.concatenate([z] * self.n_cores, axis=0), self.sharding)
                 for z in self.zero_outs]
        outs = [np.asarray(o) for o in self.fn(*dev_inputs, *zouts)]
        per_core = []
        for c in range(self.n_cores):
            d = {}
            for name, o in zip(self.out_names, outs):
                rows = o.shape[0] // self.n_cores
                d[name] = o[c * rows:(c + 1) * rows]
            per_core.append(d)
        return per_core


_CACHE = {}


def _get_runner(repeat=1, **kw):
    key = (repeat, tuple(sorted(kw.items())))
    if key not in _CACHE:
        nc = _build(repeat, **kw)
        _CACHE[key] = _SpmdRunner(nc)
    return _CACHE[key]


def _shard_inputs(emissions, tags, mask, start_transitions, end_transitions, transitions):
    import ml_dtypes
    bf = ml_dtypes.bfloat16
    em = np.ascontiguousarray(np.asarray(emissions, dtype=np.float32))
    tg = np.asarray(tags).astype(np.int32)
    mk = np.asarray(mask).astype(np.uint8)
    st = np.asarray(start_transitions, dtype=np.float32).reshape(T, 1)
    en = np.asarray(end_transitions, dtype=np.float32).reshape(T, 1)
    tr = np.ascontiguousarray(np.asarray(transitions, dtype=np.float32))
    trT = np.ascontiguousarray(tr.T)

    # host-side: one-hot match masks (sentinel 63 -> all-zero row for masked
    # steps), emb = [emissions, lastm, start-indicator], last-step weights
    tags_m = np.where(mk.astype(bool), tg, 63)                    # (B, S)
    match_full = (tags_m[:, :, None] == np.arange(T)[None, None, :]).astype(bf)
    matchS_full = np.zeros_like(match_full)
    matchS_full[:, :-1] = match_full[:, 1:]                       # shifted by one step
    mkf = mk.astype(np.float32)
    lastm = mkf.copy()
    lastm[:, :-1] -= mkf[:, 1:]                                   # 1 at s = len-1
    emb_full = np.empty((B, S, 50), dtype=bf)
    emb_full[:, :, 0:T] = em.astype(bf)
    emb_full[:, :, T] = lastm.astype(bf)
    emb_full[:, :, T + 1] = 0
    emb_full[:, 0, T + 1] = 1

    def _pair(x):
        # (BC, S, E) -> [p*BC+b, k*E+e] with s = 2k+p
        BCr, Sr, E = x.shape
        return np.ascontiguousarray(
            x.reshape(BCr, Sr // 2, 2, E).transpose(2, 0, 1, 3)).reshape(2 * BCr, (Sr // 2) * E)

    in_maps = []
    for c in range(NCORES):
        rows = slice(c * BC, (c + 1) * BC)
        em_c = em[rows]                                   # (64, S, T)
        emT32_c = np.ascontiguousarray(em_c.transpose(2, 1, 0)).reshape(T, S * BC)
        emT_c = emT32_c.astype(bf)
        mk_c = mk[rows]                                   # (64, S)
        mk1 = np.zeros_like(mkf[rows])
        mk1[:, :-1] = mkf[rows][:, 1:]
        lastw = np.ascontiguousarray((mkf[rows] - mk1).T).reshape(128, 512).astype(bf)
        in_maps.append({
            "emT": emT_c, "emT32": emT32_c,
            "matchh": _pair(match_full[rows]),
            "numrhsh": _pair(np.concatenate(
                [emb_full[rows], matchS_full[rows]], axis=2)),
            "lens": mk_c.sum(axis=1, dtype=np.float32).reshape(BC, 1),
            "lastwh": lastw,
            "transT": trT, "trans": tr, "start": st, "end": en,
        })
    return in_maps


def kernel(emissions, tags, mask, start_transitions, end_transitions, transitions):
    in_maps = _shard_inputs(emissions, tags, mask,
                            start_transitions, end_transitions, transitions)
    r = _get_runner(1)
    dev = r.put_inputs(in_maps)
    res = r(dev)
    total = np.float64(0.0)
    for c in range(NCORES):
        o = res[c]["out"][0]
        total += np.float64(o[0]) - np.float64(o[1]) - np.float64(o[2]) - np.float64(o[3]) - np.float64(o[4])
    return np.float32(total / B)



# revision 11
# speedup vs baseline: 1.5955x; 1.5955x over previous
"""CRF loss (forward-algorithm log-partition minus gold path score, batch mean)
on 8 Trainium2 NeuronCores.

Strategy (data-parallel over batch, 64 rows/core, identical SPMD program):
  Denominator via meet-in-the-middle with an augmented 49-tag state:
    forward chain over steps 0..511 and backward chain over steps 1023..512
    run concurrently (halving sequential depth, doubling chain parallelism).
    The 49th state slot absorbs masking and z-capture: host writes masked
    emissions as -60000 (exp -> exact 0) and the 49th row as +C (exp -> exact
    1), so the augmented transition F = [[M, 0], [endexp^T, 1]] captures
    z = endexp^T alpha_{L-1} into the slot the step the row finishes, and the
    backward state wakes from [0;1] at t = L-1 via the endexp injection
    column of F^T.  Final per-row z = yhat^T F ahat in one bridge matmul.
    No per-step z extraction, no rescaling (drift stays within fp32 range).
    Forward elementwise mults on DVE, backward on Pool (gpsimd).
  Numerator: one-hot match masks precomputed on host feed PSUM-accumulated
    matmuls (one fused [128c x 98] matmul per step pair): emission gather,
    bigram histogram x transitions, start/end histograms.
Host only shards/relayouts inputs and sums the 8 per-core partial scalars.
"""

import numpy as np
from contextlib import ExitStack

import concourse.bacc as bacc
import concourse.tile as tile
from concourse import mybir

B, S, T = 512, 1024, 48
TA = T + 1                # augmented tag count (48 + done-slot)
NCORES = 8
BC = B // NCORES          # rows per core = 64
HALF = S // 2             # 512 chain positions per direction
ST = 32                   # global steps per block
NBLK = HALF // ST         # 16 blocks
C_SHIFT = 4.375           # exactly representable in bf16 (keep-gates exact)
NEG = -60000.0

f32 = mybir.dt.float32
bf16 = mybir.dt.bfloat16
OP = mybir.AluOpType
AF = mybir.ActivationFunctionType


def _build(repeat=1, no_num=False, nchd=2, fu=99):
    nc = bacc.Bacc(target_bir_lowering=False, debug=False)
    # interleaved fwd/bwd emission streams: column order (s, dir, b)
    emFB_d = nc.dram_tensor("emFB", [TA, HALF * 2 * BC], bf16, kind="ExternalInput")
    # paired layouts: partition p*64+b <-> (batch b, step 2k+p); contraction
    # dim 128 so the fused numerator matmul covers TWO steps per instruction.
    # numrhs = [emb(50) | matchS(48)] -> one matmul accumulates emission/
    # start/end histograms and bigram histogram together.
    match_d = nc.dram_tensor("matchh", [2 * BC, HALF * T], bf16, kind="ExternalInput")
    numrhs_d = nc.dram_tensor("numrhsh", [2 * BC, HALF * 98], bf16, kind="ExternalInput")
    len_d = nc.dram_tensor("lens", [BC, 1], f32, kind="ExternalInput")
    lhsF_d = nc.dram_tensor("lhsF", [TA, TA], bf16, kind="ExternalInput")   # = F   (bwd chain)
    lhsG_d = nc.dram_tensor("lhsG", [TA, TA], bf16, kind="ExternalInput")   # = F^T (fwd chain + bridge)
    startmc_d = nc.dram_tensor("startmc", [TA, 1], f32, kind="ExternalInput")
    endmc_d = nc.dram_tensor("endmc", [TA, 1], f32, kind="ExternalInput")
    trans_d = nc.dram_tensor("trans", [T, T], f32, kind="ExternalInput")
    start_d = nc.dram_tensor("start", [T, 1], f32, kind="ExternalInput")
    end_d = nc.dram_tensor("end", [T, 1], f32, kind="ExternalInput")
    out_d = nc.dram_tensor("out", [1, 8], f32, kind="ExternalOutput")

    with tile.TileContext(nc) as tc, ExitStack() as ctx:
        consts = ctx.enter_context(tc.tile_pool(name="consts", bufs=1))
        rawp = ctx.enter_context(tc.tile_pool(name="rawp", bufs=2))
        dp = ctx.enter_context(tc.tile_pool(name="dp", bufs=2))
        mp = ctx.enter_context(tc.tile_pool(name="mp", bufs=2))
        nrp = ctx.enter_context(tc.tile_pool(name="nrp", bufs=2))
        ap = ctx.enter_context(tc.tile_pool(name="ap", bufs=3))
        sm = ctx.enter_context(tc.tile_pool(name="sm", bufs=2))
        cps = ctx.enter_context(tc.tile_pool(name="cps", bufs=1, space="PSUM"))
        acps = ctx.enter_context(tc.tile_pool(name="acps", bufs=1, space="PSUM"))
        tps = ctx.enter_context(tc.tile_pool(name="tps", bufs=1, space="PSUM"))

        # ---- constants ----
        lhsF = consts.tile([TA, TA], bf16)
        nc.sync.dma_start(out=lhsF, in_=lhsF_d[:, :])
        lhsG = consts.tile([TA, TA], bf16)
        nc.sync.dma_start(out=lhsG, in_=lhsG_d[:, :])
        startmc = consts.tile([TA, 1], f32)
        nc.sync.dma_start(out=startmc, in_=startmc_d[:, :])
        endmc = consts.tile([TA, 1], f32)
        nc.sync.dma_start(out=endmc, in_=endmc_d[:, :])
        trans_sb = consts.tile([T, T], f32)
        nc.sync.dma_start(out=trans_sb, in_=trans_d[:, :])
        start_sb = consts.tile([T, 1], f32)
        nc.sync.dma_start(out=start_sb, in_=start_d[:, :])
        end_sb = consts.tile([T, 1], f32)
        nc.sync.dma_start(out=end_sb, in_=end_d[:, :])
        lencol = consts.tile([BC, 1], f32)
        nc.sync.dma_start(out=lencol, in_=len_d[:, :])

        biasmc = consts.tile([TA, 1], f32)
        nc.vector.memset(biasmc, -C_SHIFT)
        b0_64 = consts.tile([BC, 1], f32)
        nc.vector.memset(b0_64, 0.0)
        ones49 = consts.tile([TA, 1], f32)
        nc.vector.memset(ones49, 1.0)
        onesP = consts.tile([BC, 1], f32)
        nc.vector.memset(onesP, 1.0)

        iota48f = consts.tile([T, T], f32)
        nc.gpsimd.iota(iota48f, pattern=[[1, T]], base=0, channel_multiplier=0,
                       allow_small_or_imprecise_dtypes=True)
        iotacolf = consts.tile([T, 1], f32)
        nc.gpsimd.iota(iotacolf, pattern=[[0, 1]], base=0, channel_multiplier=1,
                       allow_small_or_imprecise_dtypes=True)
        ident48 = consts.tile([T, T], f32)
        nc.vector.tensor_scalar(ident48, iota48f, iotacolf[:, :], None, op0=OP.is_equal)

        ws = [BC // nchd + (1 if c < BC % nchd else 0) for c in range(nchd)]
        off = [sum(ws[:c]) for c in range(nchd + 1)]

        def body(_iv):
            acc = acps.tile([T, 98], f32, tag="acc")
            accEE = acc[:, 0:50]
            accCO = acc[:, 50:98]
            if no_num:
                nc.vector.memset(acc, 1.0)
            alF = [None] * nchd
            alB = [None] * nchd

            for blk in range(NBLK):
                raw = rawp.tile([TA, ST, 2, BC], bf16, tag="raw")
                nc.sync.dma_start(out=raw, in_=emFB_d[:, blk * ST * 2 * BC:(blk + 1) * ST * 2 * BC]
                                  .rearrange("t (s d b) -> t s d b", d=2, b=BC))
                d = dp.tile([TA, ST, 2, BC], bf16, tag="d")
                nc.scalar.activation(d, raw, AF.Exp, bias=biasmc[:, :])

                if not no_num:
                    match = mp.tile([2 * BC, ST, T], bf16, tag="match")
                    nc.sync.dma_start(out=match, in_=match_d[:, blk * ST * T:(blk + 1) * ST * T]
                                      .rearrange("b (k t) -> b k t", t=T))
                    numr = nrp.tile([2 * BC, ST, 98], bf16, tag="numr")
                    nc.sync.dma_start(out=numr, in_=numrhs_d[:, blk * ST * 98:(blk + 1) * ST * 98]
                                      .rearrange("b (k e) -> b k e", e=98))

                for st in range(ST):
                    g = blk * ST + st
                    if g == 0:
                        a0 = ap.tile([TA, BC], bf16, tag="aF0i")
                        nc.scalar.activation(a0, raw[:, 0, 0, :], AF.Exp, bias=startmc[:, :])
                        alF = [a0[:, off[c]:off[c + 1]] for c in range(nchd)]
                        y0 = ap.tile([TA, BC], bf16, tag="aB0i")
                        nc.scalar.activation(y0, raw[:, 0, 1, :], AF.Exp, bias=endmc[:, :])
                        alB = [y0[:, off[c]:off[c + 1]] for c in range(nchd)]
                    else:
                        for c in range(nchd):
                            ps = cps.tile([TA, 2, ws[c]], f32, tag=f"ps{c}")
                            nc.tensor.matmul(ps[:, 0, :], lhsT=lhsG, rhs=alF[c], start=True,
                                             stop=True, skip_group_check=True)
                            nc.tensor.matmul(ps[:, 1, :], lhsT=lhsF, rhs=alB[c], start=True,
                                             stop=True, skip_group_check=True)
                            aP = ap.tile([TA, 2, ws[c]], bf16, tag=f"aP{c}")
                            nc.vector.tensor_tensor(out=aP, in0=ps,
                                                    in1=d[:, st, :, off[c]:off[c + 1]], op=OP.mult)
                            alF[c] = aP[:, 0, :]
                            alB[c] = aP[:, 1, :]

                    if not no_num:
                        nc.tensor.matmul(acc, lhsT=match[:, st, :], rhs=numr[:, st, :],
                                         start=(g == 0), stop=(g == HALF - 1),
                                         skip_group_check=True)

            # ---- finals ----
            outrow = sm.tile([1, 8], f32, tag="outrow")
            nc.vector.memset(outrow, 0.0)

            # bridge: z[b] = yhat^T F ahat = sum_k yhat[k,b] * (F ahat)[k,b]
            P = tps.tile([TA, BC], f32, tag="bridge")
            for c in range(nchd):
                nc.tensor.matmul(P[:, off[c]:off[c + 1]], lhsT=lhsG, rhs=alF[c],
                                 start=True, stop=True, skip_group_check=True)
            prod = sm.tile([TA, BC], f32, tag="prod")
            for c in range(nchd):
                nc.vector.tensor_tensor(out=prod[:, off[c]:off[c + 1]], in0=P[:, off[c]:off[c + 1]],
                                        in1=alB[c], op=OP.mult)
            zcol = tps.tile([BC, 1], f32, tag="trow")
            nc.tensor.matmul(zcol, lhsT=prod, rhs=ones49, start=True, stop=True,
                             skip_group_check=True)
            lnz = sm.tile([BC, 1], f32, tag="lnz")
            nc.scalar.activation(lnz, zcol, AF.Ln, bias=b0_64[:, :])
            logZ = sm.tile([BC, 1], f32, tag="logZ")
            nc.vector.scalar_tensor_tensor(out=logZ, in0=lencol, scalar=C_SHIFT, in1=lnz,
                                           op0=OP.mult, op1=OP.add)
            if fu <= 1:
                nc.vector.tensor_copy(outrow[0:1, 0:1], lnz[0:1, 0:1])
                nc.sync.dma_start(out=out_d[:, :], in_=outrow)
                return
            sumZ = tps.tile([1, 1], f32, tag="trow")
            nc.tensor.matmul(sumZ, lhsT=logZ, rhs=onesP, start=True, stop=True,
                             skip_group_check=True)
            nc.vector.tensor_copy(outrow[0:1, 0:1], sumZ)

            numcat = sm.tile([T, 4], f32, tag="numcat")
            nc.vector.memset(numcat, 0.0)
            trash1 = sm.tile([T, T], f32, tag="trash1")
            nc.vector.tensor_tensor(out=trash1, in0=accEE[:, 0:T], in1=ident48, op=OP.mult)
            trashb1 = sm.tile([T, T], bf16, tag="trashb1")
            nc.scalar.activation(trashb1, trash1, AF.Copy, accum_out=numcat[:, 0:1])
            trash2 = sm.tile([T, T], f32, tag="trash2")
            nc.vector.tensor_tensor(out=trash2, in0=accCO, in1=trans_sb, op=OP.mult)
            trashb2 = sm.tile([T, T], bf16, tag="trashb2")
            nc.scalar.activation(trashb2, trash2, AF.Copy, accum_out=numcat[:, 1:2])
            nc.vector.tensor_tensor(out=numcat[:, 2:3], in0=accEE[:, T:T + 1], in1=end_sb, op=OP.mult)
            nc.vector.tensor_tensor(out=numcat[:, 3:4], in0=accEE[:, T + 1:T + 2], in1=start_sb, op=OP.mult)
            ones48f = sm.tile([T, 1], f32, tag="ones48f")
            nc.vector.memset(ones48f, 1.0)
            nsum = tps.tile([1, 4], f32, tag="trow")
            nc.tensor.matmul(nsum, lhsT=ones48f, rhs=numcat, start=True, stop=True,
                             skip_group_check=True)
            nc.vector.tensor_copy(outrow[0:1, 1:5], nsum)
            nc.sync.dma_start(out=out_d[:, :], in_=outrow)

        if repeat == 1:
            body(0)
        else:
            with tc.For_i(0, repeat, 1) as iv:
                body(iv)
    nc.compile()
    return nc


class _SpmdRunner:
    def __init__(self, nc, n_cores=NCORES):
        import jax
        from jax.sharding import Mesh, PartitionSpec, NamedSharding
        from jax.experimental.shard_map import shard_map
        from concourse.bass2jax import _bass_exec_p, install_neuronx_cc_hook, partition_id_tensor
        self.jax = jax
        install_neuronx_cc_hook()
        self.nc = nc
        self.n_cores = n_cores
        partition_name = nc.partition_id_tensor.name if nc.partition_id_tensor else None
        in_names, out_names, out_avals, zero_outs = [], [], [], []
        for alloc in nc.m.functions[0].allocations:
            if not isinstance(alloc, mybir.MemoryLocationSet):
                continue
            name = alloc.memorylocations[0].name
            if alloc.kind == "ExternalInput":
                if name != partition_name:
                    in_names.append(name)
            elif alloc.kind == "ExternalOutput":
                shape = tuple(alloc.tensor_shape)
                dtype = mybir.dt.np(alloc.dtype)
                out_names.append(name)
                out_avals.append(jax.core.ShapedArray(shape, dtype))
                zero_outs.append(np.zeros(shape, dtype))
        self.in_names, self.out_names, self.zero_outs = in_names, out_names, zero_outs
        n_params, n_outs = len(in_names), len(out_avals)
        all_in = list(in_names) + list(out_names)
        if partition_name is not None:
            all_in.append(partition_name)

        def _body(*args):
            operands = list(args)
            if partition_name is not None:
                operands.append(partition_id_tensor())
            return tuple(_bass_exec_p.bind(
                *operands, out_avals=tuple(out_avals), in_names=tuple(all_in),
                out_names=tuple(out_names), lowering_input_output_aliases=(),
                sim_require_finite=True, sim_require_nnan=True, nc=nc))

        devices = jax.devices()[:n_cores]
        self.mesh = Mesh(np.asarray(devices), ("core",))
        self.fn = jax.jit(
            shard_map(_body, mesh=self.mesh,
                      in_specs=(PartitionSpec("core"),) * (n_params + n_outs),
                      out_specs=(PartitionSpec("core"),) * n_outs, check_rep=False),
            donate_argnums=tuple(range(n_params, n_params + n_outs)), keep_unused=True)
        self.sharding = NamedSharding(self.mesh, PartitionSpec("core"))

    def put_inputs(self, in_maps):
        concat = [np.concatenate([np.asarray(in_maps[c][n]) for c in range(self.n_cores)], axis=0)
                  for n in self.in_names]
        return [self.jax.device_put(a, self.sharding) for a in concat]

    def __call__(self, dev_inputs):
        zouts = [self.jax.device_put(np.concatenate([z] * self.n_cores, axis=0), self.sharding)
                 for z in self.zero_outs]
        outs = [np.asarray(o) for o in self.fn(*dev_inputs, *zouts)]
        per_core = []
        for c in range(self.n_cores):
            d = {}
            for name, o in zip(self.out_names, outs):
                rows = o.shape[0] // self.n_cores
                d[name] = o[c * rows:(c + 1) * rows]
            per_core.append(d)
        return per_core


_CACHE = {}


def _get_runner(repeat=1, **kw):
    key = (repeat, tuple(sorted(kw.items())))
    if key not in _CACHE:
        nc = _build(repeat, **kw)
        _CACHE[key] = _SpmdRunner(nc)
    return _CACHE[key]


def _shard_inputs(emissions, tags, mask, start_transitions, end_transitions, transitions):
    import ml_dtypes
    bf = ml_dtypes.bfloat16
    em = np.ascontiguousarray(np.asarray(emissions, dtype=np.float32))
    tg = np.asarray(tags).astype(np.int32)
    mk = np.asarray(mask).astype(bool)
    st = np.asarray(start_transitions, dtype=np.float32).reshape(T, 1)
    en = np.asarray(end_transitions, dtype=np.float32).reshape(T, 1)
    tr = np.ascontiguousarray(np.asarray(transitions, dtype=np.float32))

    # augmented transition F = [[exp(trans), 0], [exp(end)^T, 1]]
    F = np.zeros((TA, TA), dtype=np.float64)
    F[0:T, 0:T] = np.exp(tr.astype(np.float64))
    F[T, 0:T] = np.exp(en[:, 0].astype(np.float64))
    F[T, T] = 1.0
    lhsF = F.astype(bf)              # bwd chain lhsT
    lhsG = F.T.astype(bf)            # fwd chain lhsT (and bridge)
    startmc = np.zeros((TA, 1), dtype=np.float32)
    startmc[0:T, 0] = st[:, 0] - C_SHIFT
    endmc = np.zeros((TA, 1), dtype=np.float32)
    endmc[0:T, 0] = en[:, 0] - C_SHIFT
    endmc[T, 0] = -C_SHIFT

    # host-side: one-hot match masks (sentinel 63 -> all-zero row for masked
    # steps), emb = [emissions, lastm, start-indicator], for the numerator
    tags_m = np.where(mk, tg, 63)                                 # (B, S)
    match_full = (tags_m[:, :, None] == np.arange(T)[None, None, :]).astype(bf)
    matchS_full = np.zeros_like(match_full)
    matchS_full[:, :-1] = match_full[:, 1:]                       # shifted by one step
    mkf = mk.astype(np.float32)
    lastm = mkf.copy()
    lastm[:, :-1] -= mkf[:, 1:]                                   # 1 at s = len-1
    emb_full = np.empty((B, S, 50), dtype=bf)
    emb_full[:, :, 0:T] = em.astype(bf)
    emb_full[:, :, T] = lastm.astype(bf)
    emb_full[:, :, T + 1] = 0
    emb_full[:, 0, T + 1] = 1

    # augmented emission streams: row 48 carries the done-gate
    augE = np.where(mk[:, :, None], em, np.float32(NEG))          # (B, S, 48)
    aug48 = np.where(mk, np.float32(NEG), np.float32(C_SHIFT))    # (B, S)

    def _pair(x):
        # (BC, S, E) -> [p*BC+b, k*E+e] with s = 2k+p
        BCr, Sr, E = x.shape
        return np.ascontiguousarray(
            x.reshape(BCr, Sr // 2, 2, E).transpose(2, 0, 1, 3)).reshape(2 * BCr, (Sr // 2) * E)

    in_maps = []
    for c in range(NCORES):
        rows = slice(c * BC, (c + 1) * BC)
        full = np.empty((BC, S, TA), dtype=bf)
        full[:, :, 0:T] = augE[rows]
        full[:, :, T] = aug48[rows]
        # (dir, s, b) interleave: dir 0 = fwd step s, dir 1 = bwd step 1023-s
        fb = np.stack([full[:, 0:HALF], full[:, :HALF - 1:-1]], axis=2)  # (BC, HALF, 2, TA)
        emFB = np.ascontiguousarray(fb.transpose(3, 1, 2, 0)).reshape(TA, HALF * 2 * BC)
        in_maps.append({
            "emFB": emFB,
            "matchh": _pair(match_full[rows]),
            "numrhsh": _pair(np.concatenate(
                [emb_full[rows], matchS_full[rows]], axis=2)),
            "lens": mk[rows].sum(axis=1, dtype=np.float32).reshape(BC, 1),
            "lhsF": lhsF, "lhsG": lhsG,
            "startmc": startmc, "endmc": endmc,
            "trans": tr, "start": st, "end": en,
        })
    return in_maps


def kernel(emissions, tags, mask, start_transitions, end_transitions, transitions):
    in_maps = _shard_inputs(emissions, tags, mask,
                            start_transitions, end_transitions, transitions)
    r = _get_runner(1)
    dev = r.put_inputs(in_maps)
    res = r(dev)
    total = np.float64(0.0)
    for c in range(NCORES):
        o = res[c]["out"][0]
        total += np.float64(o[0]) - np.float64(o[1]) - np.float64(o[2]) - np.float64(o[3]) - np.float64(o[4])
    return np.float32(total / B)


# revision 27
# speedup vs baseline: 1.8935x; 1.1868x over previous
"""CRF loss (forward-algorithm log-partition minus gold path score, batch mean)
on 8 Trainium2 NeuronCores.

Strategy (data-parallel over batch, 64 rows/core, identical SPMD program):
  Denominator via meet-in-the-middle with an augmented 49-tag state:
    forward chain over steps 0..511 and backward chain over steps 1023..512
    run concurrently (halving sequential depth, doubling chain parallelism).
    The 49th state slot absorbs masking and z-capture: host writes masked
    emissions as -60000 (exp -> exact 0) and the 49th row as +C (exp -> exact
    1), so the augmented transition F = [[M, 0], [endexp^T, 1]] captures
    z = endexp^T alpha_{L-1} into the slot the step the row finishes, and the
    backward state wakes from [0;1] at t = L-1 via the endexp injection
    column of F^T.  Final per-row z = yhat^T F ahat in one bridge matmul.
    No per-step z extraction, no rescaling (drift stays within fp32 range).
    Forward elementwise mults on DVE, backward on Pool (gpsimd).
  Numerator: one-hot match masks precomputed on host feed PSUM-accumulated
    matmuls (one fused [128c x 98] matmul per step pair): emission gather,
    bigram histogram x transitions, start/end histograms.
Host only shards/relayouts inputs and sums the 8 per-core partial scalars.
"""

import numpy as np
from contextlib import ExitStack

import concourse.bacc as bacc
import concourse.tile as tile
from concourse import mybir

B, S, T = 512, 1024, 48
TA = T + 1                # augmented tag count (48 + done-slot)
TB = 64                   # bwd state base partition (32-aligned)
TA2 = 128                 # fwd state rows 0:49, bwd rows 64:113, rest zero-pad
NCORES = 8
BC = B // NCORES          # rows per core = 64
HALF = S // 2             # 512 chain positions per direction
ST = 32                   # global steps per block
NBLK = HALF // ST         # 16 blocks
C_SHIFT = 4.375           # exactly representable in bf16 (keep-gates exact)
NEG = -60000.0

f32 = mybir.dt.float32
bf16 = mybir.dt.bfloat16
OP = mybir.AluOpType
AF = mybir.ActivationFunctionType


def _build(repeat=1, no_num=False, nchd=2, fu=99):
    nc = bacc.Bacc(target_bir_lowering=False, debug=False)
    # fwd/bwd emission streams stacked on the partition axis:
    # rows 0:49 = fwd step s, rows 64:113 = bwd step 1023-s
    emFB_d = nc.dram_tensor("emFB", [TA2, HALF * BC], bf16, kind="ExternalInput")
    # paired layouts: partition p*64+b <-> (batch b, step 2k+p); contraction
    # dim 128 so the fused numerator matmul covers TWO steps per instruction.
    # numrhs = [emb(50) | matchS(48)] -> one matmul accumulates emission/
    # start/end histograms and bigram histogram together.
    match_d = nc.dram_tensor("matchh", [2 * BC, HALF * T], bf16, kind="ExternalInput")
    numrhs_d = nc.dram_tensor("numrhsh", [2 * BC, HALF * 98], bf16, kind="ExternalInput")
    len_d = nc.dram_tensor("lens", [BC, 1], f32, kind="ExternalInput")
    bd_d = nc.dram_tensor("bd", [TA2, TA2], bf16, kind="ExternalInput")     # blockdiag(F^T, F)
    lhsG_d = nc.dram_tensor("lhsG", [TA, TA], bf16, kind="ExternalInput")   # = F^T (bridge)
    startend_d = nc.dram_tensor("startend", [TA2, 1], f32, kind="ExternalInput")
    trans_d = nc.dram_tensor("trans", [T, T], f32, kind="ExternalInput")
    start_d = nc.dram_tensor("start", [T, 1], f32, kind="ExternalInput")
    end_d = nc.dram_tensor("end", [T, 1], f32, kind="ExternalInput")
    out_d = nc.dram_tensor("out", [1, 8], f32, kind="ExternalOutput")

    with tile.TileContext(nc) as tc, ExitStack() as ctx:
        consts = ctx.enter_context(tc.tile_pool(name="consts", bufs=1))
        rawp = ctx.enter_context(tc.tile_pool(name="rawp", bufs=2))
        dp = ctx.enter_context(tc.tile_pool(name="dp", bufs=2))
        mp = ctx.enter_context(tc.tile_pool(name="mp", bufs=2))
        nrp = ctx.enter_context(tc.tile_pool(name="nrp", bufs=2))
        ap = ctx.enter_context(tc.tile_pool(name="ap", bufs=3))
        sm = ctx.enter_context(tc.tile_pool(name="sm", bufs=2))
        cps = ctx.enter_context(tc.tile_pool(name="cps", bufs=1, space="PSUM"))
        acps = ctx.enter_context(tc.tile_pool(name="acps", bufs=1, space="PSUM"))
        tps = ctx.enter_context(tc.tile_pool(name="tps", bufs=1, space="PSUM"))

        # ---- constants ----
        bd = consts.tile([TA2, TA2], bf16)
        nc.sync.dma_start(out=bd, in_=bd_d[:, :])
        lhsG = consts.tile([TA, TA], bf16)
        nc.sync.dma_start(out=lhsG, in_=lhsG_d[:, :])
        startend = consts.tile([TA2, 1], f32)
        nc.sync.dma_start(out=startend, in_=startend_d[:, :])
        trans_sb = consts.tile([T, T], f32)
        nc.sync.dma_start(out=trans_sb, in_=trans_d[:, :])
        start_sb = consts.tile([T, 1], f32)
        nc.sync.dma_start(out=start_sb, in_=start_d[:, :])
        end_sb = consts.tile([T, 1], f32)
        nc.sync.dma_start(out=end_sb, in_=end_d[:, :])
        lencol = consts.tile([BC, 1], f32)
        nc.sync.dma_start(out=lencol, in_=len_d[:, :])

        biasmc = consts.tile([TA2, 1], f32)
        nc.vector.memset(biasmc, -C_SHIFT)
        b0_64 = consts.tile([BC, 1], f32)
        nc.vector.memset(b0_64, 0.0)
        ones49 = consts.tile([TA, 1], f32)
        nc.vector.memset(ones49, 1.0)
        onesP = consts.tile([BC, 1], f32)
        nc.vector.memset(onesP, 1.0)

        iota48f = consts.tile([T, T], f32)
        nc.gpsimd.iota(iota48f, pattern=[[1, T]], base=0, channel_multiplier=0,
                       allow_small_or_imprecise_dtypes=True)
        iotacolf = consts.tile([T, 1], f32)
        nc.gpsimd.iota(iotacolf, pattern=[[0, 1]], base=0, channel_multiplier=1,
                       allow_small_or_imprecise_dtypes=True)
        ident48 = consts.tile([T, T], f32)
        nc.vector.tensor_scalar(ident48, iota48f, iotacolf[:, :], None, op0=OP.is_equal)

        ws = [BC // nchd + (1 if c < BC % nchd else 0) for c in range(nchd)]
        off = [sum(ws[:c]) for c in range(nchd + 1)]

        def body(_iv):
            acc = acps.tile([T, 98], f32, tag="acc")
            accEE = acc[:, 0:50]
            accCO = acc[:, 50:98]
            if no_num:
                nc.vector.memset(acc, 1.0)
            alP = [None] * nchd

            for blk in range(NBLK):
                raw = rawp.tile([TA2, ST, BC], bf16, tag="raw")
                nc.sync.dma_start(out=raw, in_=emFB_d[:, blk * ST * BC:(blk + 1) * ST * BC]
                                  .rearrange("t (s b) -> t s b", b=BC))
                d = dp.tile([TA2, ST, BC], bf16, tag="d")
                nc.scalar.activation(d, raw, AF.Exp, bias=biasmc[:, :])

                if not no_num:
                    match = mp.tile([2 * BC, ST, T], bf16, tag="match")
                    nc.sync.dma_start(out=match, in_=match_d[:, blk * ST * T:(blk + 1) * ST * T]
                                      .rearrange("b (k t) -> b k t", t=T))
                    numr = nrp.tile([2 * BC, ST, 98], bf16, tag="numr")
                    nc.sync.dma_start(out=numr, in_=numrhs_d[:, blk * ST * 98:(blk + 1) * ST * 98]
                                      .rearrange("b (k e) -> b k e", e=98))

                for st in range(ST):
                    g = blk * ST + st
                    if g == 0:
                        a0 = ap.tile([TA2, BC], bf16, tag="a0i")
                        nc.scalar.activation(a0, raw[:, 0, :], AF.Exp, bias=startend[:, :])
                        alP = [a0[:, off[c]:off[c + 1]] for c in range(nchd)]
                        # stagger: delay chain 1+ by ~one DVE-op latency each so
                        # the pair-chains settle half a round-trip apart instead
                        # of phase-locking at a full round-trip per step
                        for c in range(1, nchd):
                            sc = ap.tile([TA2, ws[c]], bf16, tag=f"stag{c}")
                            nc.vector.tensor_copy(sc, alP[c])
                            alP[c] = sc
                    else:
                        for c in range(nchd):
                            ps = cps.tile([TA2, ws[c]], f32, tag=f"ps{c}")
                            nc.tensor.matmul(ps, lhsT=bd, rhs=alP[c], start=True,
                                             stop=True, skip_group_check=True)
                            aP = ap.tile([TA2, ws[c]], bf16, tag=f"aP{c}")
                            nc.vector.tensor_tensor(out=aP, in0=ps,
                                                    in1=d[:, st, off[c]:off[c + 1]], op=OP.mult)
                            alP[c] = aP

                    if not no_num:
                        nc.tensor.matmul(acc, lhsT=match[:, st, :], rhs=numr[:, st, :],
                                         start=(g == 0), stop=(g == HALF - 1),
                                         skip_group_check=True)

            # ---- finals ----
            outrow = sm.tile([1, 8], f32, tag="outrow")
            nc.vector.memset(outrow, 0.0)

            # bridge: z[b] = yhat^T F ahat = sum_k yhat[k,b] * (F ahat)[k,b]
            P = tps.tile([TA, BC], f32, tag="bridge")
            for c in range(nchd):
                nc.tensor.matmul(P[:, off[c]:off[c + 1]], lhsT=lhsG, rhs=alP[c][0:TA, :],
                                 start=True, stop=True, skip_group_check=True)
            prod = sm.tile([TA, BC], f32, tag="prod")
            for c in range(nchd):
                nc.vector.tensor_tensor(out=prod[:, off[c]:off[c + 1]], in0=P[:, off[c]:off[c + 1]],
                                        in1=alP[c][TB:TB + TA, :], op=OP.mult)
            zcol = tps.tile([BC, 1], f32, tag="trow")
            nc.tensor.matmul(zcol, lhsT=prod, rhs=ones49, start=True, stop=True,
                             skip_group_check=True)
            lnz = sm.tile([BC, 1], f32, tag="lnz")
            nc.scalar.activation(lnz, zcol, AF.Ln, bias=b0_64[:, :])
            logZ = sm.tile([BC, 1], f32, tag="logZ")
            nc.vector.scalar_tensor_tensor(out=logZ, in0=lencol, scalar=C_SHIFT, in1=lnz,
                                           op0=OP.mult, op1=OP.add)
            if fu <= 1:
                nc.vector.tensor_copy(outrow[0:1, 0:1], lnz[0:1, 0:1])
                nc.sync.dma_start(out=out_d[:, :], in_=outrow)
                return
            sumZ = tps.tile([1, 1], f32, tag="trow")
            nc.tensor.matmul(sumZ, lhsT=logZ, rhs=onesP, start=True, stop=True,
                             skip_group_check=True)
            nc.vector.tensor_copy(outrow[0:1, 0:1], sumZ)

            numcat = sm.tile([T, 4], f32, tag="numcat")
            nc.vector.memset(numcat, 0.0)
            trash1 = sm.tile([T, T], f32, tag="trash1")
            nc.vector.tensor_tensor(out=trash1, in0=accEE[:, 0:T], in1=ident48, op=OP.mult)
            trashb1 = sm.tile([T, T], bf16, tag="trashb1")
            nc.scalar.activation(trashb1, trash1, AF.Copy, accum_out=numcat[:, 0:1])
            trash2 = sm.tile([T, T], f32, tag="trash2")
            nc.vector.tensor_tensor(out=trash2, in0=accCO, in1=trans_sb, op=OP.mult)
            trashb2 = sm.tile([T, T], bf16, tag="trashb2")
            nc.scalar.activation(trashb2, trash2, AF.Copy, accum_out=numcat[:, 1:2])
            nc.vector.tensor_tensor(out=numcat[:, 2:3], in0=accEE[:, T:T + 1], in1=end_sb, op=OP.mult)
            nc.vector.tensor_tensor(out=numcat[:, 3:4], in0=accEE[:, T + 1:T + 2], in1=start_sb, op=OP.mult)
            ones48f = sm.tile([T, 1], f32, tag="ones48f")
            nc.vector.memset(ones48f, 1.0)
            nsum = tps.tile([1, 4], f32, tag="trow")
            nc.tensor.matmul(nsum, lhsT=ones48f, rhs=numcat, start=True, stop=True,
                             skip_group_check=True)
            nc.vector.tensor_copy(outrow[0:1, 1:5], nsum)
            nc.sync.dma_start(out=out_d[:, :], in_=outrow)

        if repeat == 1:
            body(0)
        else:
            with tc.For_i(0, repeat, 1) as iv:
                body(iv)
    nc.compile()
    return nc


class _SpmdRunner:
    def __init__(self, nc, n_cores=NCORES):
        import jax
        from jax.sharding import Mesh, PartitionSpec, NamedSharding
        from jax.experimental.shard_map import shard_map
        from concourse.bass2jax import _bass_exec_p, install_neuronx_cc_hook, partition_id_tensor
        self.jax = jax
        install_neuronx_cc_hook()
        self.nc = nc
        self.n_cores = n_cores
        partition_name = nc.partition_id_tensor.name if nc.partition_id_tensor else None
        in_names, out_names, out_avals, zero_outs = [], [], [], []
        for alloc in nc.m.functions[0].allocations:
            if not isinstance(alloc, mybir.MemoryLocationSet):
                continue
            name = alloc.memorylocations[0].name
            if alloc.kind == "ExternalInput":
                if name != partition_name:
                    in_names.append(name)
            elif alloc.kind == "ExternalOutput":
                shape = tuple(alloc.tensor_shape)
                dtype = mybir.dt.np(alloc.dtype)
                out_names.append(name)
                out_avals.append(jax.core.ShapedArray(shape, dtype))
                zero_outs.append(np.zeros(shape, dtype))
        self.in_names, self.out_names, self.zero_outs = in_names, out_names, zero_outs
        n_params, n_outs = len(in_names), len(out_avals)
        all_in = list(in_names) + list(out_names)
        if partition_name is not None:
            all_in.append(partition_name)

        def _body(*args):
            operands = list(args)
            if partition_name is not None:
                operands.append(partition_id_tensor())
            return tuple(_bass_exec_p.bind(
                *operands, out_avals=tuple(out_avals), in_names=tuple(all_in),
                out_names=tuple(out_names), lowering_input_output_aliases=(),
                sim_require_finite=True, sim_require_nnan=True, nc=nc))

        devices = jax.devices()[:n_cores]
        self.mesh = Mesh(np.asarray(devices), ("core",))
        self.fn = jax.jit(
            shard_map(_body, mesh=self.mesh,
                      in_specs=(PartitionSpec("core"),) * (n_params + n_outs),
                      out_specs=(PartitionSpec("core"),) * n_outs, check_rep=False),
            donate_argnums=tuple(range(n_params, n_params + n_outs)), keep_unused=True)
        self.sharding = NamedSharding(self.mesh, PartitionSpec("core"))

    def put_inputs(self, in_maps):
        concat = [np.concatenate([np.asarray(in_maps[c][n]) for c in range(self.n_cores)], axis=0)
                  for n in self.in_names]
        return [self.jax.device_put(a, self.sharding) for a in concat]

    def __call__(self, dev_inputs):
        zouts = [self.jax.device_put(np.concatenate([z] * self.n_cores, axis=0), self.sharding)
                 for z in self.zero_outs]
        outs = [np.asarray(o) for o in self.fn(*dev_inputs, *zouts)]
        per_core = []
        for c in range(self.n_cores):
            d = {}
            for name, o in zip(self.out_names, outs):
                rows = o.shape[0] // self.n_cores
                d[name] = o[c * rows:(c + 1) * rows]
            per_core.append(d)
        return per_core


_CACHE = {}


def _get_runner(repeat=1, **kw):
    key = (repeat, tuple(sorted(kw.items())))
    if key not in _CACHE:
        nc = _build(repeat, **kw)
        _CACHE[key] = _SpmdRunner(nc)
    return _CACHE[key]


def _shard_inputs(emissions, tags, mask, start_transitions, end_transitions, transitions):
    import ml_dtypes
    bf = ml_dtypes.bfloat16
    em = np.ascontiguousarray(np.asarray(emissions, dtype=np.float32))
    tg = np.asarray(tags).astype(np.int32)
    mk = np.asarray(mask).astype(bool)
    st = np.asarray(start_transitions, dtype=np.float32).reshape(T, 1)
    en = np.asarray(end_transitions, dtype=np.float32).reshape(T, 1)
    tr = np.ascontiguousarray(np.asarray(transitions, dtype=np.float32))

    # augmented transition F = [[exp(trans), 0], [exp(end)^T, 1]]
    F = np.zeros((TA, TA), dtype=np.float64)
    F[0:T, 0:T] = np.exp(tr.astype(np.float64))
    F[T, 0:T] = np.exp(en[:, 0].astype(np.float64))
    F[T, T] = 1.0
    lhsG = F.T.astype(bf)            # fwd chain lhsT (and bridge)
    BD = np.zeros((TA2, TA2), dtype=np.float64)
    BD[0:TA, 0:TA] = F.T             # fwd block (lhsT = F^T)
    BD[TB:TB + TA, TB:TB + TA] = F   # bwd block (lhsT = (F^T)^T = F)
    bd = BD.astype(bf)
    startend = np.zeros((TA2, 1), dtype=np.float32)
    startend[0:T, 0] = st[:, 0] - C_SHIFT
    startend[TB:TB + T, 0] = en[:, 0] - C_SHIFT
    startend[TB + T, 0] = -C_SHIFT

    # host-side: one-hot match masks (sentinel 63 -> all-zero row for masked
    # steps), emb = [emissions, lastm, start-indicator], for the numerator
    tags_m = np.where(mk, tg, 63)                                 # (B, S)
    match_full = (tags_m[:, :, None] == np.arange(T)[None, None, :]).astype(bf)
    matchS_full = np.zeros_like(match_full)
    matchS_full[:, :-1] = match_full[:, 1:]                       # shifted by one step
    mkf = mk.astype(np.float32)
    lastm = mkf.copy()
    lastm[:, :-1] -= mkf[:, 1:]                                   # 1 at s = len-1
    emb_full = np.empty((B, S, 50), dtype=bf)
    emb_full[:, :, 0:T] = em.astype(bf)
    emb_full[:, :, T] = lastm.astype(bf)
    emb_full[:, :, T + 1] = 0
    emb_full[:, 0, T + 1] = 1

    # augmented emission streams: row 48 carries the done-gate
    augE = np.where(mk[:, :, None], em, np.float32(NEG))          # (B, S, 48)
    aug48 = np.where(mk, np.float32(NEG), np.float32(C_SHIFT))    # (B, S)

    def _pair(x):
        # (BC, S, E) -> [p*BC+b, k*E+e] with s = 2k+p
        BCr, Sr, E = x.shape
        return np.ascontiguousarray(
            x.reshape(BCr, Sr // 2, 2, E).transpose(2, 0, 1, 3)).reshape(2 * BCr, (Sr // 2) * E)

    in_maps = []
    for c in range(NCORES):
        rows = slice(c * BC, (c + 1) * BC)
        full = np.empty((BC, S, TA), dtype=bf)
        full[:, :, 0:T] = augE[rows]
        full[:, :, T] = aug48[rows]
        # partition-stack: rows 0:49 = fwd step s, rows 64:113 = bwd step 1023-s
        fb = np.full((BC, HALF, TA2), NEG, dtype=bf)
        fb[:, :, 0:TA] = full[:, 0:HALF]
        fb[:, :, TB:TB + TA] = full[:, :HALF - 1:-1]
        emFB = np.ascontiguousarray(fb.transpose(2, 1, 0)).reshape(TA2, HALF * BC)
        in_maps.append({
            "emFB": emFB,
            "matchh": _pair(match_full[rows]),
            "numrhsh": _pair(np.concatenate(
                [emb_full[rows], matchS_full[rows]], axis=2)),
            "lens": mk[rows].sum(axis=1, dtype=np.float32).reshape(BC, 1),
            "bd": bd, "lhsG": lhsG, "startend": startend,
            "trans": tr, "start": st, "end": en,
        })
    return in_maps


def kernel(emissions, tags, mask, start_transitions, end_transitions, transitions):
    in_maps = _shard_inputs(emissions, tags, mask,
                            start_transitions, end_transitions, transitions)
    r = _get_runner(1)
    dev = r.put_inputs(in_maps)
    res = r(dev)
    total = np.float64(0.0)
    for c in range(NCORES):
        o = res[c]["out"][0]
        total += np.float64(o[0]) - np.float64(o[1]) - np.float64(o[2]) - np.float64(o[3]) - np.float64(o[4])
    return np.float32(total / B)
